# revision 9
# baseline (speedup 1.0000x reference)
"""BiDiTreeLSTM Trainium2 kernel.

Full-input contract: kernel(**inputs) takes the unsharded numpy inputs of
reference.setup_inputs() and returns the full [64, 512] output.

Strategy: data-parallel over trees (8 trees per NeuronCore, 8 cores).
Per-core layout is feature-major: every node-state tensor lives in SBUF as
[feature(256) = 2 partition chunks of 128, columns], where columns are
level-major blocks, tree-major within a level.  With that ordering the two
children of parent column c in level l are columns 2c and 2c+1 of level l+1,
so child gather/scatter is pure stride-2 access patterns.

All matmuls contract the feature dimension (SBUF partition axis) using
float32r (full-rate fp32 mode for moving dim >= 256).  h0/c0 are all-zero
fills per the problem spec, so the recurrence starts from zero and they are
never loaded.
"""

import numpy as np

B, NN, XS, H = 64, 1023, 256, 256
NCORES = 8
DEPTH = 9  # levels 0..9, level l has 2^l nodes per tree
TMAX = 512

_CACHE = {}

LAST_EXEC_NS = None


def _levels(bl):
    levw = [bl * (1 << l) for l in range(DEPTH + 1)]
    levo = [bl * ((1 << l) - 1) for l in range(DEPTH + 1)]
    tot = bl * NN
    return levw, levo, tot


def _build_nc(bl):
    from concourse import bacc
    import concourse.mybir as mybir
    import concourse.tile as tile

    f32 = mybir.dt.float32
    f32r = mybir.dt.float32r
    Sig = mybir.ActivationFunctionType.Sigmoid
    Tanh = mybir.ActivationFunctionType.Tanh

    LEVW, LEVO, TOT = _levels(bl)

    nc = bacc.Bacc("TRN2", target_bir_lowering=False)

    xT_d = nc.declare_dram_parameter("xT", [XS, TOT], f32, isOutput=False)
    w_iou_bu_d = nc.declare_dram_parameter("w_iou_bu_T", [XS, 3 * H], f32, isOutput=False)
    u_iou_bu_d = nc.declare_dram_parameter("u_iou_bu_T", [H, 3 * H], f32, isOutput=False)
    u_f_bu_d = nc.declare_dram_parameter("u_f_bu_T", [H, H], f32, isOutput=False)
    wx_td_d = nc.declare_dram_parameter("wx_iou_td_T", [XS, 3 * H], f32, isOutput=False)
    wh_td_d = nc.declare_dram_parameter("wh_iou_td_T", [H, 3 * H], f32, isOutput=False)
    u_iou_td_d = nc.declare_dram_parameter("u_iou_td_T", [H, 3 * H], f32, isOutput=False)
    u_f_td_d = nc.declare_dram_parameter("u_f_td_T", [H, H], f32, isOutput=False)
    b_iou_bu_d = nc.declare_dram_parameter("b_iou_bu_p", [128, 6], f32, isOutput=False)
    b_f_bu_d = nc.declare_dram_parameter("b_f_bu_p", [128, 2], f32, isOutput=False)
    b_iou_td_d = nc.declare_dram_parameter("b_iou_td_p", [128, 6], f32, isOutput=False)
    b_f_td_d = nc.declare_dram_parameter("b_f_td_p", [128, 2], f32, isOutput=False)
    out_d = nc.declare_dram_parameter("out", [512, bl], f32, isOutput=True)

    def r(ap):
        return ap.bitcast(f32r)

    with tile.TileContext(nc) as tc:
        with (
            tc.tile_pool(name="const", bufs=1) as const,
            tc.tile_pool(name="hbu_pool", bufs=1) as hbu_pool,
            tc.tile_pool(name="work", bufs=2) as work,
            tc.tile_pool(name="xtp", bufs=4) as xtp,
            tc.tile_pool(name="ps_iou", bufs=6, space="PSUM") as ps_iou,
            tc.tile_pool(name="ps_f", bufs=2, space="PSUM") as ps_f,
        ):
            # ---- weights (lhsT layout: [in_feat, out_feat]) ----
            # matmul-feeding tiles are float32r-typed so their producer
            # instructions emit fp32r-rounded values (BIR verifier rule)
            def load_mat(dram, cols, nm):
                ts = []
                for k in (0, 1):
                    t = const.tile([128, cols], f32r, name=f"{nm}{k}", tag=f"{nm}{k}")
                    nc.sync.dma_start(
                        out=t, in_=dram[k * 128:(k + 1) * 128, :].bitcast(f32r)
                    )
                    ts.append(t)
                return ts

            w_bu = load_mat(w_iou_bu_d, 3 * H, "wbu")
            u_bu = load_mat(u_iou_bu_d, 3 * H, "ubu")
            uf_bu = load_mat(u_f_bu_d, H, "ufbu")
            wx_td = load_mat(wx_td_d, 3 * H, "wxtd")
            wh_td = load_mat(wh_td_d, 3 * H, "whtd")
            u_td = load_mat(u_iou_td_d, 3 * H, "utd")
            uf_td = load_mat(u_f_td_d, H, "uftd")

            def load_bias(dram, cols, nm):
                t = const.tile([128, cols], f32, name=nm, tag=nm)
                nc.sync.dma_start(out=t, in_=dram[:, :])
                return t

            bb_iou_bu = load_bias(b_iou_bu_d, 6, "bioubu")
            bb_f_bu = load_bias(b_f_bu_d, 2, "bfbu")
            bb_iou_td = load_bias(b_iou_td_d, 6, "bioutd")
            bb_f_td = load_bias(b_f_td_d, 2, "bftd")

            # ---- persistent state ----
            hbu = [
                hbu_pool.tile([128, TOT], f32r, name=f"hbu{g}", tag=f"hbu{g}")
                for g in (0, 1)
            ]
            mean_acc = [
                const.tile([128, bl], f32, name=f"macc{g}", tag=f"macc{g}")
                for g in (0, 1)
            ]

            def load_x(off, o0, T):
                xt = []
                for k in (0, 1):
                    t = xtp.tile([128, T], f32r, name=f"xt{k}", tag=f"xt{k}")
                    nc.sync.dma_start(
                        out=t,
                        in_=xT_d[
                            k * 128:(k + 1) * 128, off + o0:off + o0 + T
                        ].bitcast(f32r),
                    )
                    xt.append(t)
                return xt

            def iou_psum(T, groups):
                """groups: list of (lhsT_pair, rhs_pair) accumulated per m-chunk.
                Returns 6 psum tiles [128, T]."""
                pg = []
                for m in range(6):
                    p = ps_iou.tile([128, T], f32, name="pg", tag="pg")
                    mms = []
                    for lhs_pair, rhs_pair in groups:
                        for k in (0, 1):
                            mms.append(
                                (lhs_pair[k][:, m * 128:(m + 1) * 128], rhs_pair[k])
                            )
                    n = len(mms)
                    for i, (lhs, rhs) in enumerate(mms):
                        nc.tensor.matmul(
                            p, r(lhs), r(rhs), start=(i == 0), stop=(i == n - 1)
                        )
                    pg.append(p)
                return pg

            def gates(pg, bb, T, c_red, c_out, h_out, leaf_sink=None):
                """c_red: None (zero) or list of 2 [128, T]-broadcastable APs.
                c_out[g]: AP to write c; h_out[g]: AP to write h (or None with
                leaf_sink callback receiving (g, h_tile))."""
                for g in (0, 1):
                    si = work.tile([128, T], f32, name="si", tag="ga", bufs=4)
                    nc.scalar.activation(si, pg[g], Sig, bias=bb[:, g:g + 1])
                    tu = work.tile([128, T], f32, name="tu", tag="gb", bufs=4)
                    nc.scalar.activation(tu, pg[4 + g], Tanh, bias=bb[:, 4 + g:5 + g])
                    cs = c_out[g]
                    if c_red is None:
                        nc.vector.tensor_mul(cs, si, tu)
                    else:
                        nc.vector.tensor_mul(si, si, tu)
                        cr = c_red[g]
                        if cr.shape != tuple(si.shape):
                            # broadcast parent -> children (pairs of columns)
                            nc.vector.tensor_add(
                                cs.rearrange("p (n two) -> p n two", two=2),
                                si.rearrange("p (n two) -> p n two", two=2),
                                cr,
                            )
                        else:
                            nc.vector.tensor_add(cs, si, cr)
                    tc_t = work.tile([128, T], f32, name="tc_t", tag="ga", bufs=4)
                    nc.scalar.activation(tc_t, cs, Tanh)
                    so = work.tile([128, T], f32, name="so", tag="gb", bufs=4)
                    nc.scalar.activation(so, pg[2 + g], Sig, bias=bb[:, 2 + g:3 + g])
                    if h_out is not None:
                        nc.vector.tensor_mul(h_out[g], so, tc_t)
                    else:
                        ht = work.tile([128, T], f32, name="ht", tag="hsum", bufs=2)
                        nc.vector.tensor_mul(ht, so, tc_t)
                        leaf_sink(g, ht)

            # ================= bottom-up =================
            with tc.tile_pool(name="bu_state", bufs=1) as bu_state:
                c_next = None
                for l in range(DEPTH, -1, -1):
                    C, off = LEVW[l], LEVO[l]
                    T = min(TMAX, C)
                    par = "A" if l % 2 else "Bp"
                    c_cur = [
                        bu_state.tile(
                            [128, C], f32, name=f"c{l}_{g}", tag=f"c{par}{g}"
                        )
                        for g in (0, 1)
                    ]
                    leaf = l == DEPTH
                    choff = LEVO[l + 1] if not leaf else None
                    for j in range(C // T):
                        o0 = j * T
                        xt = load_x(off, o0, T)
                        cred = None
                        hsum = None
                        if not leaf:
                            ncj = 2 if 2 * T > TMAX else 1
                            Wc = 2 * T // ncj
                            fc = [[None, None] for _ in range(ncj)]
                            for cj in range(ncj):
                                cbase = choff + 2 * o0 + cj * Wc
                                for g in (0, 1):
                                    pf = ps_f.tile([128, Wc], f32, name="pf", tag="pf")
                                    for k in (0, 1):
                                        nc.tensor.matmul(
                                            pf,
                                            r(uf_bu[k][:, g * 128:(g + 1) * 128]),
                                            r(hbu[k][:, cbase:cbase + Wc]),
                                            start=(k == 0),
                                            stop=(k == 1),
                                        )
                                    nc.scalar.activation(
                                        pf, pf, Sig, bias=bb_f_bu[:, g:g + 1]
                                    )
                                    fct = work.tile(
                                        [128, Wc], f32, name="fct", tag="fc", bufs=5
                                    )
                                    nc.vector.tensor_mul(
                                        fct,
                                        pf,
                                        c_next[g][:, 2 * o0 + cj * Wc:2 * o0 + (cj + 1) * Wc],
                                    )
                                    fc[cj][g] = fct
                            cred, hsum = [], []
                            half = T // ncj
                            for g in (0, 1):
                                cr = work.tile([128, T], f32, name="cr", tag="cred", bufs=3)
                                hs = work.tile([128, T], f32r, name="hs", tag="hsum", bufs=2)
                                for cj in range(ncj):
                                    seg = fc[cj][g]
                                    nc.vector.tensor_add(
                                        cr[:, cj * half:(cj + 1) * half],
                                        seg[:, 0::2],
                                        seg[:, 1::2],
                                    )
                                    hb = hbu[g][:, choff + 2 * o0 + cj * Wc:choff + 2 * o0 + (cj + 1) * Wc]
                                    nc.vector.tensor_add(
                                        hs[:, cj * half:(cj + 1) * half],
                                        hb[:, 0::2],
                                        hb[:, 1::2],
                                    )
                                cred.append(cr)
                                hsum.append(hs)
                        groups = [(w_bu, xt)]
                        if not leaf:
                            groups.append((u_bu, hsum))
                        pg = iou_psum(T, groups)
                        gates(
                            pg,
                            bb_iou_bu,
                            T,
                            cred,
                            [c_cur[g][:, o0:o0 + T] for g in (0, 1)],
                            [hbu[g][:, off + o0:off + o0 + T] for g in (0, 1)],
                        )
                    c_next = c_cur

            # ================= top-down =================
            with tc.tile_pool(name="td_state", bufs=1) as td_state:
                h_prev = c_prev = None
                for l in range(0, DEPTH + 1):
                    C, off = LEVW[l], LEVO[l]
                    T = min(TMAX, C)
                    leaf = l == DEPTH
                    root = l == 0
                    par = "A" if l % 2 else "Bp"
                    if not leaf:
                        h_cur = [
                            td_state.tile([128, C], f32r, name=f"th{l}_{g}", tag=f"th{par}{g}")
                            for g in (0, 1)
                        ]
                        c_cur = [
                            td_state.tile([128, C], f32, name=f"tc{l}_{g}", tag=f"tc{par}{g}")
                            for g in (0, 1)
                        ]
                    else:
                        h_cur = c_cur = None
                    for j in range(C // T):
                        o0 = j * T
                        xt = load_x(off, o0, T)
                        credp = None
                        if not root:
                            pT = T // 2
                            po = o0 // 2
                            credp = []
                            for g in (0, 1):
                                pf = ps_f.tile([128, pT], f32, name="pftd", tag="pf")
                                for k in (0, 1):
                                    nc.tensor.matmul(
                                        pf,
                                        r(uf_td[k][:, g * 128:(g + 1) * 128]),
                                        r(h_prev[k][:, po:po + pT]),
                                        start=(k == 0),
                                        stop=(k == 1),
                                    )
                                nc.scalar.activation(pf, pf, Sig, bias=bb_f_td[:, g:g + 1])
                                cr = work.tile([128, pT], f32, name="crtd", tag="cred", bufs=3)
                                nc.vector.tensor_mul(cr, pf, c_prev[g][:, po:po + pT])
                                credp.append(cr.to_broadcast([128, pT, 2]))
                        groups = [
                            (wx_td, xt),
                            (wh_td, [hbu[k][:, off + o0:off + o0 + T] for k in (0, 1)]),
                        ]
                        if not root:
                            groups.append(
                                (
                                    u_td,
                                    [
                                        h_prev[k][:, po:po + pT].to_broadcast([128, pT, 2])
                                        for k in (0, 1)
                                    ],
                                )
                            )
                        pg = iou_psum(T, groups)
                        if leaf:
                            ct = [
                                work.tile([128, T], f32, name=f"ctl{g}", tag="cred", bufs=3)
                                for g in (0, 1)
                            ]
                            tree = (off + o0 - LEVO[DEPTH]) // (1 << DEPTH) if bl > 1 else 0

                            def sink(g, ht, _j=j):
                                # leaf tile j covers exactly one tree's leaves
                                # when T == 512; for bl == 1 it is tree 0.
                                nc.vector.reduce_sum(
                                    mean_acc[g][:, _j:_j + 1],
                                    ht,
                                    axis=mybir.AxisListType.X,
                                )

                            gates(pg, bb_iou_td, T, credp, ct, None, leaf_sink=sink)
                        else:
                            gates(
                                pg,
                                bb_iou_td,
                                T,
                                credp,
                                [c_cur[g][:, o0:o0 + T] for g in (0, 1)],
                                [h_cur[g][:, o0:o0 + T] for g in (0, 1)],
                            )
                    h_prev, c_prev = h_cur, c_cur

            # ---- outputs ----
            nleaf = LEVW[DEPTH] // max(1, LEVW[DEPTH] // TMAX)  # leaves per tile
            for g in (0, 1):
                nc.vector.tensor_scalar_mul(mean_acc[g], mean_acc[g], 1.0 / (1 << DEPTH))
            nc.sync.dma_start(out=out_d[0:128, :].bitcast(f32r), in_=hbu[0][:, 0:bl])
            nc.sync.dma_start(out=out_d[128:256, :].bitcast(f32r), in_=hbu[1][:, 0:bl])
            nc.sync.dma_start(out=out_d[256:384, :], in_=mean_acc[0])
            nc.sync.dma_start(out=out_d[384:512, :], in_=mean_acc[1])

    # Bacc defers register allocation etc. to finalize(); the raw
    # run_bass_via_pjrt path does not call it for us.
    if not nc.is_finalized():
        nc.finalize()
    return nc


def _prep_shared(inputs):
    """Weight/bias marshaling shared by all cores."""
    f = np.ascontiguousarray

    def packb(b):
        return f(b.reshape(-1, 128).T.astype(np.float32))

    W_iou_td = np.asarray(inputs["W_iou_td"], np.float32)
    return {
        "w_iou_bu_T": f(np.asarray(inputs["W_iou_bu"], np.float32).T),
        "u_iou_bu_T": f(np.asarray(inputs["U_iou_bu"], np.float32).T),
        "u_f_bu_T": f(np.asarray(inputs["U_f_bu"], np.float32).T),
        "wx_iou_td_T": f(W_iou_td[:, :XS].T),
        "wh_iou_td_T": f(W_iou_td[:, XS:].T),
        "u_iou_td_T": f(np.asarray(inputs["U_iou_td"], np.float32).T),
        "u_f_td_T": f(np.asarray(inputs["U_f_td"], np.float32).T),
        "b_iou_bu_p": packb(np.asarray(inputs["b_iou_bu"], np.float32)),
        "b_f_bu_p": packb(np.asarray(inputs["b_f_bu"], np.float32)),
        "b_iou_td_p": packb(np.asarray(inputs["b_iou_td"], np.float32)),
        "b_f_td_p": packb(np.asarray(inputs["b_f_td"], np.float32)),
    }


def prep_xt(Xc):
    """[bl, NN, XS] -> [XS, bl*NN] with level-major column blocks."""
    bl = Xc.shape[0]
    xt = np.asarray(Xc, np.float32).transpose(2, 0, 1)  # [XS, bl, NN]
    blocks = []
    for l in range(DEPTH + 1):
        lo, nl = (1 << l) - 1, 1 << l
        blocks.append(xt[:, :, lo:lo + nl].reshape(XS, bl * nl))
    return np.ascontiguousarray(np.concatenate(blocks, axis=1))


def unpack_out(o, bl):
    """[512, bl] -> [bl, 512] (root_h_bu | leaf mean)."""
    return np.concatenate([o[0:256, :].T, o[256:512, :].T], axis=1)


def kernel(**inputs):
    global LAST_EXEC_NS
    from concourse.bass_utils import run_bass_kernel_spmd

    bl = B // NCORES
    if "nc" not in _CACHE:
        _CACHE["nc"] = _build_nc(bl)
    nc = _CACHE["nc"]

    shared = _prep_shared(inputs)
    X = np.asarray(inputs["X"], np.float32)
    in_maps = []
    for c in range(NCORES):
        m = dict(shared)
        m["xT"] = prep_xt(X[c * bl:(c + 1) * bl])
        in_maps.append(m)

    trace = _CACHE.get("trace", False)
    res = run_bass_kernel_spmd(nc, in_maps, list(range(NCORES)), trace=trace)
    LAST_EXEC_NS = res.exec_time_ns
    _CACHE["last_results"] = res

    out = np.concatenate(
        [unpack_out(res.results[c]["out"], bl) for c in range(NCORES)], axis=0
    )
    return out.astype(np.float32)
